# revision 10
# baseline (speedup 1.0000x reference)
"""Trainium2 Bass kernel for nn_BenchCADecoder (cellular-automaton decoder).

Model: x = embed[tokens]+pos; rw = softmax(gate*1e-3 @ sel_w + sel_b) (step
invariant); 5 CA steps of x = LN(x + sum_r rw[t,r] * MLP_r([x, roll(x,1),
roll(x,-1)])); out = LN_f(x) @ head_w.

Sharding: pure data-parallel over (batch, T-half): 8 cores x 1024 tokens,
each with a circular halo of HALO tokens per side so the 5 neighbor-coupled
steps need zero inter-core communication (halo shrinks by 1 per step).

On-chip layout: x kept transposed [D, tok] so roll() is a free-dim slice
shift and both MLP matmuls run with the contraction on partitions. The
rule-weighted sum is folded into PSUM accumulation of the second matmul by
pre-scaling gelu outputs with broadcast rule weights. LayerNorm stats
(partition-dim reductions) via ones-vector matmuls. All matmuls in
float32r (FP22): full PE rate at free-dim >= 256, ~6e-5 relative precision.
"""

import os
import sys
from contextlib import ExitStack

import numpy as np

sys.path.insert(0, "/opt/trn_rl_repo")

import concourse.bacc as bacc
import concourse.bass as bass
import concourse.mybir as mybir
import concourse.tile as tile
from concourse.bass import IndirectOffsetOnAxis
from concourse.bass_utils import run_bass_kernel_spmd
from concourse.masks import make_identity

F32 = mybir.dt.float32
F32R = mybir.dt.float32r
I32 = mybir.dt.int32
AF = mybir.ActivationFunctionType
OP = mybir.AluOpType

P = 128


class Cfg:
    def __init__(self, D=512, R=8, V=32000, T=2048, B=4, steps=5, own=1024,
                 halo=6, eps=1e-5, gate_scale=1e-3, newton=True):
        self.D, self.R, self.V, self.T, self.B = D, R, V, T, B
        self.steps, self.own, self.halo = steps, own, halo
        self.eps, self.gate_scale = eps, gate_scale
        self.newton = newton
        self.DC = D // P                 # d chunks
        self.HID = 2 * D
        self.HC = self.HID // P          # hidden chunks
        self.KC = 3 * self.DC            # contraction chunks for mm1
        self.WIN = own + 2 * halo        # active token window
        self.NT9 = (self.WIN + P - 1) // P   # gather tiles
        self.WBUF = self.NT9 * P + 2     # x buffer cols (pad col each side)
        # token tiles (col offset, width), cols 1..WIN active
        tt = []
        c = 1
        while c <= self.WIN:
            w = min(512, self.WIN - c + 1)
            tt.append((c, w))
            c += w
        self.tok_tiles = tt
        # head vocab tiles
        vt = []
        v = 0
        while v < V:
            w = min(512, V - v)
            vt.append((v, w))
            v += w
        self.v_tiles = vt
        self.n_tok_chunks = own // P     # head token chunks (128 each)
        self.own_col0 = 1 + halo         # first owned col in x buffer


def _r(ap):
    """View an f32 AP as float32r for PE consumption."""
    return ap.bitcast(F32R)


def build_nc(cfg: Cfg, num_devices=8):
    """Build the single-core (SPMD) Bass module."""
    nc = bacc.Bacc("TRN2", target_bir_lowering=False, debug=False,
                   num_devices=num_devices)
    D, R, V, DC, HC, KC = cfg.D, cfg.R, cfg.V, cfg.DC, cfg.HC, cfg.KC
    S = cfg.steps

    # ---- DRAM I/O ------------------------------------------------------
    toksT = nc.dram_tensor("toksT", [P, cfg.NT9], I32, kind="ExternalInput").ap()
    gate = nc.dram_tensor("gate", [cfg.NT9 * P, D], F32, kind="ExternalInput").ap()
    pos = nc.dram_tensor("pos", [cfg.NT9 * P, D], F32, kind="ExternalInput").ap()
    embed = nc.dram_tensor("embed", [V, D], F32, kind="ExternalInput").ap()
    w1t = nc.dram_tensor("w1t", [R, HC, P, KC, P], F32R, kind="ExternalInput").ap()
    b1 = nc.dram_tensor("b1", [R, 2 * D], F32, kind="ExternalInput").ap()
    w2t = nc.dram_tensor("w2t", [R, HC, P, DC, P], F32R, kind="ExternalInput").ap()
    b2 = nc.dram_tensor("b2", [R, D], F32R, kind="ExternalInput").ap()
    selw = nc.dram_tensor("selw", [D, R], F32R, kind="ExternalInput").ap()
    selb = nc.dram_tensor("selb", [1, R], F32R, kind="ExternalInput").ap()
    ng = nc.dram_tensor("ng", [S, D], F32, kind="ExternalInput").ap()
    nb_ = nc.dram_tensor("nb", [S, D], F32, kind="ExternalInput").ap()
    lg = nc.dram_tensor("lg", [1, D], F32, kind="ExternalInput").ap()
    lb = nc.dram_tensor("lb", [1, D], F32, kind="ExternalInput").ap()
    headw = nc.dram_tensor("headw", [D, V], F32R, kind="ExternalInput").ap()
    ones1_d = nc.dram_tensor("ones1", [P, 1], F32R, kind="ExternalInput").ap()
    ones8_d = nc.dram_tensor("ones8d", [8, P], F32R, kind="ExternalInput").ap()
    xz = nc.dram_tensor("xz", [P, cfg.DC, 1], F32R, kind="ExternalInput").ap()
    out = nc.dram_tensor("out", [cfg.own, V], F32, kind="ExternalOutput").ap()

    with ExitStack() as ctx:
        ctx.enter_context(nc.allow_low_precision(reason="f32r is fp32-width"))
        tc = ctx.enter_context(tile.TileContext(nc))
        _emit(ctx, tc, cfg, toksT, gate, pos, embed, w1t, b1, w2t, b2, selw,
              selb, ng, nb_, lg, lb, headw, out, ones1_d, ones8_d, xz)
    nc.compile()
    return nc


def _emit(ctx, tc, cfg, toksT, gate, pos, embed, w1t, b1, w2t, b2, selw,
          selb, ng, nb_, lg, lb, headw, out, ones1_d, ones8_d, xz):
    nc = tc.nc
    D, R, V, DC, HC, KC = cfg.D, cfg.R, cfg.V, cfg.DC, cfg.HC, cfg.KC
    S, WIN, NT9, WBUF = cfg.steps, cfg.WIN, cfg.NT9, cfg.WBUF

    def mm(o, lh, rh, start, stop):
        nc.tensor.matmul(o, _r(lh), _r(rh), start=start, stop=stop)

    # ---- persistent SBUF ----------------------------------------------
    persist = ctx.enter_context(tc.tile_pool(name="persist", bufs=1))
    xA = persist.tile([P, DC, WBUF], F32R, name="xA")
    xB = persist.tile([P, DC, WBUF], F32R, name="xB")
    rwB = persist.tile([P, R, WIN + 2], F32R, name="rwB")      # bcast rule weights
    rwT = persist.tile([R, NT9 * P], F32R, name="rwT")      # rw [r, tok]
    ident = persist.tile([P, P], F32, name="ident")
    ones128 = persist.tile([P, 1], F32R, name="ones128")
    ones8 = persist.tile([8, P], F32R, name="ones8")
    b1_sb = persist.tile([P, R, HC], F32, name="b1_sb")
    b2_sb = persist.tile([R, DC, P], F32R, name="b2_sb")
    ng_sb = persist.tile([P, S, DC], F32, name="ng_sb")
    nbv_sb = persist.tile([P, S, DC], F32, name="nbv_sb")
    lg_sb = persist.tile([P, 1, DC], F32, name="lg_sb")
    lb_sb = persist.tile([P, 1, DC], F32, name="lb_sb")

    make_identity(nc, ident)
    nc.sync.dma_start(out=ones128, in_=ones1_d)
    nc.sync.dma_start(out=ones8[0:8, :], in_=ones8_d)
    for xb_ in (xA, xB):   # zero the pad columns read by shifted slices
        nc.sync.dma_start(out=xb_[:, :, 0:1], in_=xz)
        nc.sync.dma_start(out=xb_[:, :, 1 + WIN:2 + WIN], in_=xz)

    nc.sync.dma_start(out=b1_sb, in_=bass.AP(
        b1.tensor, 0, [[1, P], [2 * D, R], [P, HC]]))
    nc.sync.dma_start(out=b2_sb, in_=bass.AP(
        b2.tensor, 0, [[D, R], [P, DC], [1, P]]))
    nc.sync.dma_start(out=ng_sb, in_=bass.AP(
        ng.tensor, 0, [[1, P], [D, S], [P, DC]]))
    nc.sync.dma_start(out=nbv_sb, in_=bass.AP(
        nb_.tensor, 0, [[1, P], [D, S], [P, DC]]))
    nc.sync.dma_start(out=lg_sb, in_=bass.AP(
        lg.tensor, 0, [[1, P], [D, 1], [P, DC]]))
    nc.sync.dma_start(out=lb_sb, in_=bass.AP(
        lb.tensor, 0, [[1, P], [D, 1], [P, DC]]))

    # ---- setup: embed gather + pos -> xA (transposed); gate -> rw ------
    with tc.tile_pool(name="setup", bufs=3) as sp, \
         tc.tile_pool(name="setup_ps", bufs=2, space="PSUM") as spp, \
         tc.tile_pool(name="setup_small", bufs=2) as ss:
        idx = persist.tile([P, NT9], I32, name="idx")
        nc.sync.dma_start(out=idx, in_=toksT)
        selw_sb = persist.tile([P, DC, R], F32R, name="selw_sb")
        nc.sync.dma_start(out=selw_sb, in_=bass.AP(
            selw.tensor, 0, [[R, P], [P * R, DC], [1, R]]))
        nc.vector.tensor_scalar_mul(selw_sb, selw_sb, cfg.gate_scale)
        selb_sb = persist.tile([1, R], F32R, name="selb_sb")
        nc.sync.dma_start(out=selb_sb, in_=selb)

        for i in range(NT9):
            # x tile: gather embed rows + pos
            xg = sp.tile([P, D], F32, tag="xg")
            nc.gpsimd.indirect_dma_start(
                out=xg, out_offset=None, in_=embed,
                in_offset=IndirectOffsetOnAxis(ap=idx[:, i:i + 1], axis=0))
            pt = sp.tile([P, D], F32, tag="pt")
            nc.sync.dma_start(out=pt, in_=pos[i * P:(i + 1) * P, :])
            nc.vector.tensor_add(xg, xg, pt)
            tp = spp.tile([P, DC, P], F32, space="PSUM", tag="tp")
            for dc in range(DC):
                nc.tensor.transpose(tp[:, dc, :], xg[:, dc * P:(dc + 1) * P], ident)
            nc.vector.tensor_copy(xA[:, :, 1 + i * P:1 + (i + 1) * P], tp)

            # gate tile -> gateT (transposed), then logits -> rw
            gt = sp.tile([P, D], F32, tag="gt")
            nc.sync.dma_start(out=gt, in_=gate[i * P:(i + 1) * P, :])
            tg = spp.tile([P, DC, P], F32, space="PSUM", tag="tp")
            for dc in range(DC):
                nc.tensor.transpose(tg[:, dc, :], gt[:, dc * P:(dc + 1) * P], ident)
            gT = sp.tile([P, DC, P], F32R, tag="gT")
            nc.vector.tensor_copy(gT, tg)

            lp = spp.tile([P, R], F32, space="PSUM", tag="lp")
            for dc in range(DC):
                mm(lp, gT[:, dc, :], selw_sb[:, dc, :], dc == 0, False)
            mm(lp, ones8[0:1, :], selb_sb, False, True)  # rank-1 +sel_b
            e = ss.tile([P, R], F32, tag="e")
            nc.scalar.activation(e, lp, AF.Exp)
            esum = ss.tile([P, 1], F32, tag="es")
            nc.vector.tensor_reduce(esum, e, mybir.AxisListType.X, OP.add)
            nc.vector.reciprocal(esum, esum)
            nc.vector.tensor_scalar(out=e, in0=e, scalar1=esum, scalar2=None,
                                    op0=OP.mult)
            rp = spp.tile([R, P], F32, space="PSUM", tag="rp")
            nc.tensor.transpose(rp, e, ident)
            nc.vector.tensor_copy(rwT[:, i * P:(i + 1) * P], rp)

        # broadcast rw rows across partitions: rwB[p, r, c] = rw[tok c-1, r]
        # (bounce via DRAM: SBUF sources cannot have partition step 0)
        rw_dram = nc.dram_tensor("rw_scratch", [R, NT9 * P], F32R).ap()
        nc.sync.dma_start(out=rw_dram, in_=rwT)
        for r in range(R):
            nc.sync.dma_start(
                out=rwB[:, r, 1:1 + WIN],
                in_=bass.AP(rw_dram.tensor, r * NT9 * P, [[0, P], [1, WIN]]))

    # ---- CA steps ------------------------------------------------------
    with tc.tile_pool(name="w1p", bufs=3) as wp, \
         tc.tile_pool(name="w2p", bufs=3) as w2p, \
         tc.tile_pool(name="gp", bufs=2) as gp, \
         tc.tile_pool(name="rbp", bufs=2) as rp_, \
         tc.tile_pool(name="rowp", bufs=2) as rowp, \
         tc.tile_pool(name="evp", bufs=1, space="PSUM") as evp, \
         tc.tile_pool(name="hpp", bufs=2, space="PSUM") as hpp, \
         tc.tile_pool(name="stp", bufs=1, space="PSUM") as stp:

        def layer_norm(xc, xn, c0, nt, ev, g_col, b_col):
            """LN of (xc[:, :, c0:c0+nt] + ev) -> xn cols; ev may be None."""
            inv_d = 1.0 / D
            if ev is not None:
                rb = rp_.tile([P, DC, nt], F32R, tag="rb")
                nc.vector.tensor_add(rb, xc[:, :, c0:c0 + nt], ev)
            else:
                rb = xc[:, :, c0:c0 + nt]
            sq = rp_.tile([P, DC, nt], F32R, tag="sq")
            nc.vector.tensor_mul(sq, rb, rb)
            st_s = stp.tile([1, nt], F32, space="PSUM", tag="sts")
            st_q = stp.tile([1, nt], F32, space="PSUM", tag="stq")
            for dc in range(DC):
                mm(st_s, ones128, rb[:, dc, :], dc == 0, dc == DC - 1)
            for dc in range(DC):
                mm(st_q, ones128, sq[:, dc, :], dc == 0, dc == DC - 1)
            mrow = rowp.tile([1, nt], F32, tag="mrow")
            nc.vector.tensor_scalar_mul(mrow, st_s, inv_d)
            msq = rowp.tile([1, nt], F32, tag="msq")
            nc.vector.tensor_mul(msq, mrow, mrow)
            wrow = rowp.tile([1, nt], F32, tag="wrow")
            # wrow = st_q/D - m^2 (+eps)
            nc.vector.scalar_tensor_tensor(out=wrow, in0=st_q, scalar=inv_d,
                                           in1=msq, op0=OP.mult,
                                           op1=OP.subtract)
            nc.vector.tensor_scalar_add(wrow, wrow, cfg.eps)
            srow = rowp.tile([1, nt], F32R, tag="srow")
            nc.scalar.activation(srow, wrow, AF.Sqrt)
            nc.vector.reciprocal(srow, srow)
            if cfg.newton:  # one Newton step: s *= 1.5 - 0.5*w*s*s
                t1 = rowp.tile([1, nt], F32, tag="msq", name="t1")
                nc.vector.tensor_mul(t1, wrow, srow)
                nc.vector.tensor_mul(t1, t1, srow)
                nc.vector.tensor_scalar(out=t1, in0=t1, scalar1=-0.5,
                                        scalar2=1.5, op0=OP.mult, op1=OP.add)
                nc.vector.tensor_mul(srow, srow, t1)
            # nms = -m*s
            nms = rowp.tile([1, nt], F32R, tag="nms")
            nc.vector.scalar_tensor_tensor(out=nms, in0=mrow, scalar=-1.0,
                                           in1=srow, op0=OP.mult, op1=OP.mult)
            bc = evp.tile([P, 2, nt], F32, space="PSUM", tag="ev")
            mm(bc[:, 0, :], ones8[0:1, :], srow, True, True)
            mm(bc[:, 1, :], ones8[0:1, :], nms, True, True)
            u = rp_.tile([P, DC, nt], F32, tag="sq", name="u")
            nc.vector.tensor_mul(u, rb, bc[:, 0:1, :].broadcast_to([P, DC, nt]))
            nc.vector.tensor_add(u, u, bc[:, 1:2, :].broadcast_to([P, DC, nt]))
            for dc in range(DC):
                nc.vector.tensor_scalar(
                    out=xn[:, dc, c0:c0 + nt], in0=u[:, dc, :],
                    scalar1=g_col[:, dc:dc + 1], scalar2=b_col[:, dc:dc + 1],
                    op0=OP.mult, op1=OP.add)

        for s in range(S):
            xc, xn = (xA, xB) if s % 2 == 0 else (xB, xA)
            for (c0, nt) in cfg.tok_tiles:
                ev = evp.tile([P, DC, nt], F32, space="PSUM", tag="ev")
                for dc in range(DC):  # seed: sum_r rw[t,r]*b2[r,d]
                    mm(ev[:, dc, :], b2_sb[:, dc, :],
                       rwT[:, c0 - 1:c0 - 1 + nt], True, False)

                def emit_mm2(gtile, r, hc, last):
                    w2_sb = w2p.tile([P, DC, P], F32R, tag="w2")
                    nc.sync.dma_start(out=w2_sb, in_=w2t[r, hc])
                    for dc in range(DC):
                        mm(ev[:, dc, :], w2_sb[:, dc, :], gtile,
                           False, last and dc == DC - 1)

                prev = None  # mm2 pipelined one hc behind mm1
                for r in range(R):
                    for hc in range(HC):
                        w1_sb = wp.tile([P, KC, P], F32R, tag="w1")
                        nc.sync.dma_start(out=w1_sb, in_=w1t[r, hc])
                        hp = hpp.tile([P, nt], F32, space="PSUM", tag="hp")
                        for kg, sh in enumerate((0, -1, 1)):
                            for kd in range(DC):
                                kc = kg * DC + kd
                                mm(hp, w1_sb[:, kc, :],
                                   xc[:, kd, c0 + sh:c0 + sh + nt],
                                   kc == 0, kc == KC - 1)
                        gtile = gp.tile([P, nt], F32R, tag="g")
                        nc.scalar.activation(gtile, hp, AF.Gelu,
                                             bias=b1_sb[:, r, hc:hc + 1])
                        nc.vector.tensor_mul(gtile, gtile,
                                             rwB[:, r, c0:c0 + nt])
                        if prev is not None:
                            emit_mm2(*prev, False)
                        prev = (gtile, r, hc)
                emit_mm2(*prev, True)
                layer_norm(xc, xn, c0, nt, ev,
                           ng_sb[:, s, :], nbv_sb[:, s, :])

        # ---- final LN --------------------------------------------------
        xc, xf = (xA, xB) if S % 2 == 0 else (xB, xA)
        for (c0, nt) in cfg.tok_tiles:
            layer_norm(xc, xf, c0, nt, None, lg_sb[:, 0, :], lb_sb[:, 0, :])

    # ---- head ----------------------------------------------------------
    with tc.tile_pool(name="hwp", bufs=3) as hwp, \
         tc.tile_pool(name="obp", bufs=4) as obp, \
         tc.tile_pool(name="outp", bufs=6, space="PSUM") as outp:
        copy_i = 0
        for (v0, vn) in cfg.v_tiles:
            hw_sb = hwp.tile([P, DC, vn], F32R, tag="hw")
            nc.sync.dma_start(out=hw_sb, in_=bass.AP(
                headw.tensor, v0, [[V, P], [P * V, DC], [1, vn]]))
            for tk in range(cfg.n_tok_chunks):
                c = cfg.own_col0 + tk * P
                op = outp.tile([P, vn], F32, space="PSUM", tag="op")
                for dc in range(DC):
                    mm(op, xf[:, dc, c:c + P], hw_sb[:, dc, :],
                       dc == 0, dc == DC - 1)
                ob = obp.tile([P, vn], F32, tag="ob")
                if copy_i % 2 == 0:
                    nc.vector.tensor_copy(ob, op)
                else:
                    nc.scalar.copy(ob, op)
                copy_i += 1
                nc.sync.dma_start(out=out[tk * P:(tk + 1) * P, v0:v0 + vn],
                                  in_=ob)


# ---- host-side sharding / unsharding -----------------------------------

def shard_inputs(cfg: Cfg, tokens, gate_signal, embed, pos_embed, rule_w1,
                 rule_b1, rule_w2, rule_b2, sel_w, sel_b, norm_g, norm_b,
                 lnf_g, lnf_b, head_w, n_cores=8):
    D, R, V, T, B = cfg.D, cfg.R, cfg.V, cfg.T, cfg.B
    w1t = np.ascontiguousarray(
        np.asarray(rule_w1, np.float32)
        .reshape(R, cfg.KC, P, cfg.HC, P).transpose(0, 3, 2, 1, 4))
    w2t = np.ascontiguousarray(
        np.asarray(rule_w2, np.float32).reshape(R, cfg.HC, P, cfg.DC, P))
    shared = {
        "embed": np.ascontiguousarray(embed, np.float32),
        "w1t": w1t,
        "b1": np.ascontiguousarray(rule_b1, np.float32),
        "w2t": w2t,
        "b2": np.ascontiguousarray(rule_b2, np.float32),
        "selw": np.ascontiguousarray(sel_w, np.float32),
        "selb": np.ascontiguousarray(sel_b, np.float32).reshape(1, R),
        "ng": np.ascontiguousarray(norm_g, np.float32),
        "nb": np.ascontiguousarray(norm_b, np.float32),
        "lg": np.ascontiguousarray(lnf_g, np.float32).reshape(1, D),
        "lb": np.ascontiguousarray(lnf_b, np.float32).reshape(1, D),
        "headw": np.ascontiguousarray(head_w, np.float32),
        "ones1": np.ones((P, 1), np.float32),
        "ones8d": np.ones((8, P), np.float32),
        "xz": np.zeros((P, cfg.DC, 1), np.float32),
    }
    halves = T // cfg.own
    in_maps = []
    for c in range(n_cores):
        b, h = divmod(c, halves)
        t0 = h * cfg.own
        w = np.arange(t0 - cfg.halo, t0 - cfg.halo + cfg.NT9 * P) % T
        toks_win = np.asarray(tokens)[b, w].astype(np.int32)
        m = dict(shared)
        m["toksT"] = np.ascontiguousarray(toks_win.reshape(cfg.NT9, P).T)
        m["gate"] = np.ascontiguousarray(
            np.asarray(gate_signal, np.float32)[0, w, :])
        m["pos"] = np.ascontiguousarray(np.asarray(pos_embed, np.float32)[w, :])
        in_maps.append(m)
    return in_maps


def unshard_output(cfg: Cfg, results, n_cores=8):
    halves = cfg.T // cfg.own
    out = np.empty((cfg.B, cfg.T, cfg.V), np.float32)
    for c in range(n_cores):
        b, h = divmod(c, halves)
        out[b, h * cfg.own:(h + 1) * cfg.own, :] = results[c]["out"]
    return out


_NC_CACHE = {}


def kernel(**inputs):
    cfg = Cfg()
    if "full" not in _NC_CACHE:
        _NC_CACHE["full"] = build_nc(cfg)
    nc = _NC_CACHE["full"]
    in_maps = shard_inputs(cfg, **{k: np.asarray(v) for k, v in inputs.items()})
    res = run_bass_kernel_spmd(nc, in_maps, core_ids=list(range(8)))
    return unshard_output(cfg, res.results)


# revision 14
# speedup vs baseline: 22.6734x; 22.6734x over previous
"""Trainium2 Bass kernel for nn_BenchCADecoder (cellular-automaton decoder).

Model: x = embed[tokens]+pos; rw = softmax(gate*1e-3 @ sel_w + sel_b) (step
invariant); 5 CA steps of x = LN(x + sum_r rw[t,r] * MLP_r([x, roll(x,1),
roll(x,-1)])); out = LN_f(x) @ head_w.

Sharding: pure data-parallel over (batch, T-half): 8 cores x 1024 tokens,
each with a circular halo of HALO tokens per side so the 5 neighbor-coupled
steps need zero inter-core communication (halo shrinks by 1 per step).

On-chip layout: x kept transposed [D, tok] so roll() is a free-dim slice
shift and both MLP matmuls run with the contraction on partitions. The
rule-weighted sum is folded into PSUM accumulation of the second matmul by
pre-scaling gelu outputs with broadcast rule weights. LayerNorm stats
(partition-dim reductions) via ones-vector matmuls. All matmuls in
float32r (FP22): full PE rate at free-dim >= 256, ~6e-5 relative precision.
"""

import os
import sys
from contextlib import ExitStack

import numpy as np

sys.path.insert(0, "/opt/trn_rl_repo")

import concourse.bacc as bacc
import concourse.bass as bass
import concourse.mybir as mybir
import concourse.tile as tile
from concourse.bass import IndirectOffsetOnAxis
from concourse.bass_utils import run_bass_kernel_spmd
from concourse.masks import make_identity

F32 = mybir.dt.float32
F32R = mybir.dt.float32r
I32 = mybir.dt.int32
AF = mybir.ActivationFunctionType
OP = mybir.AluOpType

P = 128


class Cfg:
    def __init__(self, D=512, R=8, V=32000, T=2048, B=4, steps=5, own=1024,
                 halo=6, eps=1e-5, gate_scale=1e-3, newton=True):
        self.D, self.R, self.V, self.T, self.B = D, R, V, T, B
        self.steps, self.own, self.halo = steps, own, halo
        self.eps, self.gate_scale = eps, gate_scale
        self.newton = newton
        self.DC = D // P                 # d chunks
        self.HID = 2 * D
        self.HC = self.HID // P          # hidden chunks
        self.KC = 3 * self.DC            # contraction chunks for mm1
        self.WIN = own + 2 * halo        # active token window
        self.NT9 = (self.WIN + P - 1) // P   # gather tiles
        self.WBUF = self.NT9 * P + 2     # x buffer cols (pad col each side)
        # token tiles (col offset, width), cols 1..WIN active
        tt = []
        c = 1
        while c <= self.WIN:
            w = min(512, self.WIN - c + 1)
            tt.append((c, w))
            c += w
        self.tok_tiles = tt
        # head vocab tiles
        vt = []
        v = 0
        while v < V:
            w = min(512, V - v)
            vt.append((v, w))
            v += w
        self.v_tiles = vt
        self.n_tok_chunks = own // P     # head token chunks (128 each)
        self.own_col0 = 1 + halo         # first owned col in x buffer


def _r(ap):
    """View an f32 AP as float32r for PE consumption."""
    return ap.bitcast(F32R)


def build_nc(cfg: Cfg, num_devices=8):
    """Build the single-core (SPMD) Bass module."""
    nc = bacc.Bacc("TRN2", target_bir_lowering=False, debug=False,
                   num_devices=num_devices)
    D, R, V, DC, HC, KC = cfg.D, cfg.R, cfg.V, cfg.DC, cfg.HC, cfg.KC
    S = cfg.steps

    # ---- DRAM I/O ------------------------------------------------------
    toksT = nc.dram_tensor("toksT", [P, cfg.NT9], I32, kind="ExternalInput").ap()
    gate = nc.dram_tensor("gate", [cfg.NT9 * P, D], F32, kind="ExternalInput").ap()
    pos = nc.dram_tensor("pos", [cfg.NT9 * P, D], F32, kind="ExternalInput").ap()
    embed = nc.dram_tensor("embed", [V, D], F32, kind="ExternalInput").ap()
    w1t = nc.dram_tensor("w1t", [R, HC, P, KC, P], F32R, kind="ExternalInput").ap()
    b1 = nc.dram_tensor("b1", [R, 2 * D], F32, kind="ExternalInput").ap()
    w2t = nc.dram_tensor("w2t", [R, HC, P, DC, P], F32R, kind="ExternalInput").ap()
    b2 = nc.dram_tensor("b2", [R, D], F32R, kind="ExternalInput").ap()
    selw = nc.dram_tensor("selw", [D, R], F32R, kind="ExternalInput").ap()
    selb = nc.dram_tensor("selb", [1, R], F32R, kind="ExternalInput").ap()
    ng = nc.dram_tensor("ng", [S, D], F32, kind="ExternalInput").ap()
    nb_ = nc.dram_tensor("nb", [S, D], F32, kind="ExternalInput").ap()
    lg = nc.dram_tensor("lg", [1, D], F32, kind="ExternalInput").ap()
    lb = nc.dram_tensor("lb", [1, D], F32, kind="ExternalInput").ap()
    headw = nc.dram_tensor("headw", [D, V], F32R, kind="ExternalInput").ap()
    ones1_d = nc.dram_tensor("ones1", [P, 1], F32R, kind="ExternalInput").ap()
    ones8_d = nc.dram_tensor("ones8d", [8, P], F32R, kind="ExternalInput").ap()
    xz = nc.dram_tensor("xz", [P, cfg.DC, 1], F32R, kind="ExternalInput").ap()
    out = nc.dram_tensor("out", [cfg.own, V], F32, kind="ExternalOutput").ap()

    with ExitStack() as ctx:
        ctx.enter_context(nc.allow_low_precision(reason="f32r is fp32-width"))
        tc = ctx.enter_context(tile.TileContext(nc))
        _emit(ctx, tc, cfg, toksT, gate, pos, embed, w1t, b1, w2t, b2, selw,
              selb, ng, nb_, lg, lb, headw, out, ones1_d, ones8_d, xz)
    nc.compile()
    return nc


def _emit(ctx, tc, cfg, toksT, gate, pos, embed, w1t, b1, w2t, b2, selw,
          selb, ng, nb_, lg, lb, headw, out, ones1_d, ones8_d, xz):
    nc = tc.nc
    D, R, V, DC, HC, KC = cfg.D, cfg.R, cfg.V, cfg.DC, cfg.HC, cfg.KC
    S, WIN, NT9, WBUF = cfg.steps, cfg.WIN, cfg.NT9, cfg.WBUF

    def mm(o, lh, rh, start, stop):
        nc.tensor.matmul(o, _r(lh), _r(rh), start=start, stop=stop)

    # ---- persistent SBUF ----------------------------------------------
    persist = ctx.enter_context(tc.tile_pool(name="persist", bufs=1))
    xA = persist.tile([P, DC, WBUF], F32R, name="xA")
    xB = persist.tile([P, DC, WBUF], F32R, name="xB")
    rwB = persist.tile([P, R, WIN + 2], F32R, name="rwB")      # bcast rule weights
    rwT = persist.tile([R, NT9 * P], F32R, name="rwT")      # rw [r, tok]
    ident = persist.tile([P, P], F32, name="ident")
    ones128 = persist.tile([P, 1], F32R, name="ones128")
    ones8 = persist.tile([8, P], F32R, name="ones8")
    b1_sb = persist.tile([P, R, HC], F32, name="b1_sb")
    b2_sb = persist.tile([R, DC, P], F32R, name="b2_sb")
    ng_sb = persist.tile([P, S, DC], F32, name="ng_sb")
    nbv_sb = persist.tile([P, S, DC], F32, name="nbv_sb")
    lg_sb = persist.tile([P, 1, DC], F32, name="lg_sb")
    lb_sb = persist.tile([P, 1, DC], F32, name="lb_sb")

    make_identity(nc, ident)
    nc.sync.dma_start(out=ones128, in_=ones1_d)
    nc.sync.dma_start(out=ones8[0:8, :], in_=ones8_d)
    for xb_ in (xA, xB):   # zero the pad columns read by shifted slices
        nc.sync.dma_start(out=xb_[:, :, 0:1], in_=xz)
        nc.sync.dma_start(out=xb_[:, :, 1 + WIN:2 + WIN], in_=xz)

    nc.sync.dma_start(out=b1_sb, in_=bass.AP(
        b1.tensor, 0, [[1, P], [2 * D, R], [P, HC]]))
    nc.sync.dma_start(out=b2_sb, in_=bass.AP(
        b2.tensor, 0, [[D, R], [P, DC], [1, P]]))
    nc.sync.dma_start(out=ng_sb, in_=bass.AP(
        ng.tensor, 0, [[1, P], [D, S], [P, DC]]))
    nc.sync.dma_start(out=nbv_sb, in_=bass.AP(
        nb_.tensor, 0, [[1, P], [D, S], [P, DC]]))
    nc.sync.dma_start(out=lg_sb, in_=bass.AP(
        lg.tensor, 0, [[1, P], [D, 1], [P, DC]]))
    nc.sync.dma_start(out=lb_sb, in_=bass.AP(
        lb.tensor, 0, [[1, P], [D, 1], [P, DC]]))

    # ---- setup: embed gather + pos -> xA (transposed); gate -> rw ------
    with tc.tile_pool(name="setup", bufs=3) as sp, \
         tc.tile_pool(name="setup_ps", bufs=2, space="PSUM") as spp, \
         tc.tile_pool(name="setup_small", bufs=2) as ss:
        idx = persist.tile([P, NT9], I32, name="idx")
        nc.sync.dma_start(out=idx, in_=toksT)
        selw_sb = persist.tile([P, DC, R], F32R, name="selw_sb")
        nc.sync.dma_start(out=selw_sb, in_=bass.AP(
            selw.tensor, 0, [[R, P], [P * R, DC], [1, R]]))
        nc.vector.tensor_scalar_mul(selw_sb, selw_sb, cfg.gate_scale)
        selb_sb = persist.tile([1, R], F32R, name="selb_sb")
        nc.sync.dma_start(out=selb_sb, in_=selb)

        for i in range(NT9):
            # x tile: gather embed rows + pos
            xg = sp.tile([P, D], F32, tag="xg")
            nc.gpsimd.indirect_dma_start(
                out=xg, out_offset=None, in_=embed,
                in_offset=IndirectOffsetOnAxis(ap=idx[:, i:i + 1], axis=0))
            pt = sp.tile([P, D], F32, tag="pt")
            nc.sync.dma_start(out=pt, in_=pos[i * P:(i + 1) * P, :])
            nc.vector.tensor_add(xg, xg, pt)
            tp = spp.tile([P, DC, P], F32, space="PSUM", tag="tp")
            for dc in range(DC):
                nc.tensor.transpose(tp[:, dc, :], xg[:, dc * P:(dc + 1) * P], ident)
            nc.vector.tensor_copy(xA[:, :, 1 + i * P:1 + (i + 1) * P], tp)

            # gate tile -> gateT (transposed), then logits -> rw
            gt = sp.tile([P, D], F32, tag="gt")
            nc.sync.dma_start(out=gt, in_=gate[i * P:(i + 1) * P, :])
            tg = spp.tile([P, DC, P], F32, space="PSUM", tag="tp")
            for dc in range(DC):
                nc.tensor.transpose(tg[:, dc, :], gt[:, dc * P:(dc + 1) * P], ident)
            gT = sp.tile([P, DC, P], F32R, tag="gT")
            nc.vector.tensor_copy(gT, tg)

            lp = spp.tile([P, R], F32, space="PSUM", tag="lp")
            for dc in range(DC):
                mm(lp, gT[:, dc, :], selw_sb[:, dc, :], dc == 0, False)
            mm(lp, ones8[0:1, :], selb_sb, False, True)  # rank-1 +sel_b
            e = ss.tile([P, R], F32, tag="e")
            nc.scalar.activation(e, lp, AF.Exp)
            esum = ss.tile([P, 1], F32, tag="es")
            nc.vector.tensor_reduce(esum, e, mybir.AxisListType.X, OP.add)
            nc.vector.reciprocal(esum, esum)
            nc.vector.tensor_scalar(out=e, in0=e, scalar1=esum, scalar2=None,
                                    op0=OP.mult)
            rp = spp.tile([R, P], F32, space="PSUM", tag="rp")
            nc.tensor.transpose(rp, e, ident)
            nc.vector.tensor_copy(rwT[:, i * P:(i + 1) * P], rp)

        # broadcast rw rows across partitions: rwB[p, r, c] = rw[tok c-1, r]
        # (bounce via DRAM: SBUF sources cannot have partition step 0)
        rw_dram = nc.dram_tensor("rw_scratch", [R, NT9 * P], F32R).ap()
        nc.sync.dma_start(out=rw_dram, in_=rwT)
        for r in range(R):
            nc.sync.dma_start(
                out=rwB[:, r, 1:1 + WIN],
                in_=bass.AP(rw_dram.tensor, r * NT9 * P, [[0, P], [1, WIN]]))

    # ---- CA steps ------------------------------------------------------
    with tc.tile_pool(name="w1p", bufs=3) as wp, \
         tc.tile_pool(name="w2p", bufs=2) as w2p, \
         tc.tile_pool(name="g8p", bufs=1) as g8p, \
         tc.tile_pool(name="evsp", bufs=1) as evsp, \
         tc.tile_pool(name="rbp", bufs=1) as rp_, \
         tc.tile_pool(name="rowp", bufs=1) as rowp, \
         tc.tile_pool(name="evp", bufs=1, space="PSUM") as evp, \
         tc.tile_pool(name="hpp", bufs=2, space="PSUM") as hpp, \
         tc.tile_pool(name="stp", bufs=1, space="PSUM") as stp:

        def layer_norm(xc, xn, c0, nt, ev, g_col, b_col):
            """LN of (xc[:, :, c0:c0+nt] + ev) -> xn cols; ev may be None."""
            inv_d = 1.0 / D
            if ev is not None:
                rb = rp_.tile([P, DC, nt], F32R, tag="rb")
                nc.vector.tensor_add(rb, xc[:, :, c0:c0 + nt], ev)
            else:
                rb = xc[:, :, c0:c0 + nt]
            sq = rp_.tile([P, DC, nt], F32R, tag="sq")
            nc.vector.tensor_mul(sq, rb, rb)
            st_s = stp.tile([1, nt], F32, space="PSUM", tag="sts")
            st_q = stp.tile([1, nt], F32, space="PSUM", tag="stq")
            for dc in range(DC):
                mm(st_s, ones128, rb[:, dc, :], dc == 0, dc == DC - 1)
            for dc in range(DC):
                mm(st_q, ones128, sq[:, dc, :], dc == 0, dc == DC - 1)
            mrow = rowp.tile([1, nt], F32, tag="mrow")
            nc.vector.tensor_scalar_mul(mrow, st_s, inv_d)
            msq = rowp.tile([1, nt], F32, tag="msq")
            nc.vector.tensor_mul(msq, mrow, mrow)
            wrow = rowp.tile([1, nt], F32, tag="wrow")
            # wrow = st_q/D - m^2 (+eps)
            nc.vector.scalar_tensor_tensor(out=wrow, in0=st_q, scalar=inv_d,
                                           in1=msq, op0=OP.mult,
                                           op1=OP.subtract)
            nc.vector.tensor_scalar_add(wrow, wrow, cfg.eps)
            srow = rowp.tile([1, nt], F32R, tag="srow")
            nc.scalar.activation(srow, wrow, AF.Sqrt)
            nc.vector.reciprocal(srow, srow)
            if cfg.newton:  # one Newton step: s *= 1.5 - 0.5*w*s*s
                t1 = rowp.tile([1, nt], F32, tag="msq", name="t1")
                nc.vector.tensor_mul(t1, wrow, srow)
                nc.vector.tensor_mul(t1, t1, srow)
                nc.vector.tensor_scalar(out=t1, in0=t1, scalar1=-0.5,
                                        scalar2=1.5, op0=OP.mult, op1=OP.add)
                nc.vector.tensor_mul(srow, srow, t1)
            # nms = -m*s
            nms = rowp.tile([1, nt], F32R, tag="nms")
            nc.vector.scalar_tensor_tensor(out=nms, in0=mrow, scalar=-1.0,
                                           in1=srow, op0=OP.mult, op1=OP.mult)
            bc = evp.tile([P, 2, nt], F32, space="PSUM", tag="ev")
            mm(bc[:, 0, :], ones8[0:1, :], srow, True, True)
            mm(bc[:, 1, :], ones8[0:1, :], nms, True, True)
            u = rp_.tile([P, DC, nt], F32, tag="sq", name="u")
            nc.vector.tensor_mul(u, rb, bc[:, 0:1, :].broadcast_to([P, DC, nt]))
            nc.vector.tensor_add(u, u, bc[:, 1:2, :].broadcast_to([P, DC, nt]))
            for dc in range(DC):
                nc.vector.tensor_scalar(
                    out=xn[:, dc, c0:c0 + nt], in0=u[:, dc, :],
                    scalar1=g_col[:, dc:dc + 1], scalar2=b_col[:, dc:dc + 1],
                    op0=OP.mult, op1=OP.add)

        for s in range(S):
            xc, xn = (xA, xB) if s % 2 == 0 else (xB, xA)
            # evolved accumulates in SBUF across rules so weights stream once
            evs = evsp.tile([P, DC, WIN + 2], F32, tag="evs")
            for r in range(R):
                g8 = g8p.tile([P, HC, WIN + 2], F32R, tag="g8")
                for hc in range(HC):
                    w1_sb = wp.tile([P, KC, P], F32R, tag="w1")
                    nc.sync.dma_start(out=w1_sb, in_=w1t[r, hc])
                    for (c0, nt) in cfg.tok_tiles:
                        hp = hpp.tile([P, nt], F32, space="PSUM", tag="hp")
                        for kg, sh in enumerate((0, -1, 1)):
                            for kd in range(DC):
                                kc = kg * DC + kd
                                mm(hp, w1_sb[:, kc, :],
                                   xc[:, kd, c0 + sh:c0 + sh + nt],
                                   kc == 0, kc == KC - 1)
                        nc.scalar.activation(g8[:, hc, c0:c0 + nt], hp,
                                             AF.Gelu,
                                             bias=b1_sb[:, r, hc:hc + 1])
                        nc.vector.tensor_mul(g8[:, hc, c0:c0 + nt],
                                             g8[:, hc, c0:c0 + nt],
                                             rwB[:, r, c0:c0 + nt])
                w2r = w2p.tile([P, HC, DC, P], F32R, tag="w2")
                nc.scalar.dma_start(out=w2r, in_=w2t[r].transpose([1, 0, 2, 3]))
                for (c0, nt) in cfg.tok_tiles:
                    evp_ps = evp.tile([P, DC, nt], F32, space="PSUM", tag="ev")
                    # start=True zeroes the whole 2KB psum bank: when several
                    # dc regions share a bank (narrow runt tile), only the
                    # first region in each bank may carry start, and only the
                    # last may carry stop.
                    bank = [(dc * nt * 4) // 2048 for dc in range(DC)]
                    first_b = [dc == 0 or bank[dc] != bank[dc - 1]
                               for dc in range(DC)]
                    last_b = [dc == DC - 1 or bank[dc + 1] != bank[dc]
                              for dc in range(DC)]
                    if r == 0:  # seed: sum_r rw[t,r]*b2[r,d]
                        for dc in range(DC):
                            mm(evp_ps[:, dc, :], b2_sb[:, dc, :],
                               rwT[:, c0 - 1:c0 - 1 + nt], first_b[dc], False)
                    for hc in range(HC):
                        for dc in range(DC):
                            mm(evp_ps[:, dc, :], w2r[:, hc, dc, :],
                               g8[:, hc, c0:c0 + nt],
                               r > 0 and hc == 0 and first_b[dc],
                               hc == HC - 1 and last_b[dc])
                    if r == 0:
                        nc.vector.tensor_copy(evs[:, :, c0:c0 + nt], evp_ps)
                    else:
                        nc.vector.tensor_add(evs[:, :, c0:c0 + nt],
                                             evs[:, :, c0:c0 + nt], evp_ps)
            for (c0, nt) in cfg.tok_tiles:
                layer_norm(xc, xn, c0, nt, evs[:, :, c0:c0 + nt],
                           ng_sb[:, s, :], nbv_sb[:, s, :])

        # ---- final LN --------------------------------------------------
        xc, xf = (xA, xB) if S % 2 == 0 else (xB, xA)
        for (c0, nt) in cfg.tok_tiles:
            layer_norm(xc, xf, c0, nt, None, lg_sb[:, 0, :], lb_sb[:, 0, :])

    # ---- head ----------------------------------------------------------
    with tc.tile_pool(name="hwp", bufs=3) as hwp, \
         tc.tile_pool(name="obp", bufs=4) as obp, \
         tc.tile_pool(name="outp", bufs=6, space="PSUM") as outp:
        copy_i = 0
        for (v0, vn) in cfg.v_tiles:
            hw_sb = hwp.tile([P, DC, vn], F32R, tag="hw")
            nc.sync.dma_start(out=hw_sb, in_=bass.AP(
                headw.tensor, v0, [[V, P], [P * V, DC], [1, vn]]))
            for tk in range(cfg.n_tok_chunks):
                c = cfg.own_col0 + tk * P
                op = outp.tile([P, vn], F32, space="PSUM", tag="op")
                for dc in range(DC):
                    mm(op, xf[:, dc, c:c + P], hw_sb[:, dc, :],
                       dc == 0, dc == DC - 1)
                ob = obp.tile([P, vn], F32, tag="ob")
                if copy_i % 2 == 0:
                    nc.vector.tensor_copy(ob, op)
                else:
                    nc.scalar.copy(ob, op)
                copy_i += 1
                nc.sync.dma_start(out=out[tk * P:(tk + 1) * P, v0:v0 + vn],
                                  in_=ob)


# ---- host-side sharding / unsharding -----------------------------------

def shard_inputs(cfg: Cfg, tokens, gate_signal, embed, pos_embed, rule_w1,
                 rule_b1, rule_w2, rule_b2, sel_w, sel_b, norm_g, norm_b,
                 lnf_g, lnf_b, head_w, n_cores=8):
    D, R, V, T, B = cfg.D, cfg.R, cfg.V, cfg.T, cfg.B
    w1t = np.ascontiguousarray(
        np.asarray(rule_w1, np.float32)
        .reshape(R, cfg.KC, P, cfg.HC, P).transpose(0, 3, 2, 1, 4))
    w2t = np.ascontiguousarray(
        np.asarray(rule_w2, np.float32).reshape(R, cfg.HC, P, cfg.DC, P))
    shared = {
        "embed": np.ascontiguousarray(embed, np.float32),
        "w1t": w1t,
        "b1": np.ascontiguousarray(rule_b1, np.float32),
        "w2t": w2t,
        "b2": np.ascontiguousarray(rule_b2, np.float32),
        "selw": np.ascontiguousarray(sel_w, np.float32),
        "selb": np.ascontiguousarray(sel_b, np.float32).reshape(1, R),
        "ng": np.ascontiguousarray(norm_g, np.float32),
        "nb": np.ascontiguousarray(norm_b, np.float32),
        "lg": np.ascontiguousarray(lnf_g, np.float32).reshape(1, D),
        "lb": np.ascontiguousarray(lnf_b, np.float32).reshape(1, D),
        "headw": np.ascontiguousarray(head_w, np.float32),
        "ones1": np.ones((P, 1), np.float32),
        "ones8d": np.ones((8, P), np.float32),
        "xz": np.zeros((P, cfg.DC, 1), np.float32),
    }
    halves = T // cfg.own
    in_maps = []
    for c in range(n_cores):
        b, h = divmod(c, halves)
        t0 = h * cfg.own
        w = np.arange(t0 - cfg.halo, t0 - cfg.halo + cfg.NT9 * P) % T
        toks_win = np.asarray(tokens)[b, w].astype(np.int32)
        m = dict(shared)
        m["toksT"] = np.ascontiguousarray(toks_win.reshape(cfg.NT9, P).T)
        m["gate"] = np.ascontiguousarray(
            np.asarray(gate_signal, np.float32)[0, w, :])
        m["pos"] = np.ascontiguousarray(np.asarray(pos_embed, np.float32)[w, :])
        in_maps.append(m)
    return in_maps


def unshard_output(cfg: Cfg, results, n_cores=8):
    halves = cfg.T // cfg.own
    out = np.empty((cfg.B, cfg.T, cfg.V), np.float32)
    for c in range(n_cores):
        b, h = divmod(c, halves)
        out[b, h * cfg.own:(h + 1) * cfg.own, :] = results[c]["out"]
    return out


_NC_CACHE = {}


def kernel(**inputs):
    cfg = Cfg()
    if "full" not in _NC_CACHE:
        _NC_CACHE["full"] = build_nc(cfg)
    nc = _NC_CACHE["full"]
    in_maps = shard_inputs(cfg, **{k: np.asarray(v) for k, v in inputs.items()})
    res = run_bass_kernel_spmd(nc, in_maps, core_ids=list(range(8)))
    return unshard_output(cfg, res.results)
